# revision 14
# baseline (speedup 1.0000x reference)
"""GAT edge->relation aggregation (nn_GAT_E_to_R) on 8 Trainium2 NeuronCores.

Strategy (relation-sharded SPMD, zero collectives):
  - Core c owns relations [125c, 125(c+1)). All edges of a relation live on
    one core, so segment-softmax denominators and segment sums are fully
    local to one core.
  - Phase 1 (device): project x_e with augmented weights (bf16 matmuls; x_e
    arrives host-transposed so the contraction dim is on partitions) to build
    bf16 node tables tab[row] = [proj (128) | 1.0 | s_a | s_b | junk] with
    512B rows in DRAM. Row 0 is all-zero; node n lives at row n+1.
  - Phase 2 (device): per chunk of 32 edge tiles (128 edges each), dma_gather
    pulls 512B table rows for heads (tabH[h+1]) and tails (tabT[t+1]).
    Scores use the embedded s columns: ex = exp(leaky_relu(s1[h]+s2[t])).
    Two PE matmuls per tile per stream: lhsT=rows[:,0:128] (feats),
    rhs=ex[128,1] accumulates sum_e ex_e*feat_e into PSUM *column* slot(rel)
    of U_T [128f x 125slots]; lhsT=rows[:,128:129] (the 1.0 column)
    accumulates the softmax denominator z into z_T [1 x 125]. Pad edges
    gather an all-zero row, contributing nothing to either.
  - Finalize: out[slot] = (U1_T/z1 + U2_T/z2), transposed via one PE
    identity-matmul per stream.

dma_gather indices are int16 (< 32768), so each slot's edges are split into
4 classes by (head row >= 32768, tail row >= 32768); each gather call
addresses one table half via a base-offset view. Zero rows exist in both
halves (row 0 and rows > N) for padding.

The tile -> PSUM-column schedule (and matmul start/stop flags) is static and
identical on all 8 cores: per (class, slot), the tile budget is the max over
cores. Per-core data (indices) fills the shared schedule; surplus tiles
gather zero rows into the junk PSUM column 126.

Softmax max-subtraction is skipped: scores are O(1) (|e| < ~3), exp cannot
overflow, and alpha = exp(e)/sum exp(e) is unchanged by the shift.
"""

import os
import numpy as np

# Problem dims (hardcoded per harness contract).
N_NODES = 50000
E_EDGES = 1600000
E_HIDDEN = 256
R_HIDDEN = 128
R_RELS = 1000
N_CORES = 8
R_PER_CORE = R_RELS // N_CORES  # 125

ROW = 256          # table row (bf16): 128 feats | 1.0 | s_a | s_b | junk
C_ONE = 128        # ones column
C_SA = 129         # score scalar a
C_SB = 130         # score scalar b
HALF = 32768       # int16 index range per gather call
T_C = 32           # edge tiles per phase-2 chunk (tile = 128 edges)
P1_TILES = 8       # node tiles per phase-1 chunk (tile = 128 nodes)
JUNK_SLOT = 126    # PSUM column for junk tiles

_CACHE = {}
LAST_RUN_INFO = {}


def _build_program(n_pad, n_chunks, chunk_hlo, chunk_tlo,
                   slot_of_tile, start_flag, stop_flag):
    """Build + compile the (shared, SPMD) bass program. Returns compiled nc."""
    import concourse.bacc as bacc
    import concourse.tile as tile
    from concourse import mybir
    from concourse.masks import make_identity

    f32 = mybir.dt.float32
    bf16 = mybir.dt.bfloat16
    i16 = mybir.dt.int16
    ALU = mybir.AluOpType
    ACTF = mybir.ActivationFunctionType

    n_tiles_pad = len(slot_of_tile)
    assert n_tiles_pad == n_chunks * T_C
    IDXW = (T_C * 128) // 16  # idx free width per chunk (16-wrapped)
    GN = min(1024, T_C * 128)  # indices per dma_gather call
    GSUB = (T_C * 128) // GN   # sub-calls per chunk

    nc = bacc.Bacc(
        "TRN2",
        target_bir_lowering=False,
        debug=False,
        enable_asserts=False,
        num_devices=N_CORES,
    )

    x_eT_d = nc.dram_tensor("x_eT", [E_HIDDEN, n_pad], bf16, kind="ExternalInput")
    wcat0_d = nc.dram_tensor("wcat0", [128, 264], bf16, kind="ExternalInput")
    wcat1_d = nc.dram_tensor("wcat1", [128, 264], bf16, kind="ExternalInput")
    idxh_d = nc.dram_tensor("idx_h", [n_chunks, 128, IDXW], i16, kind="ExternalInput")
    idxt_d = nc.dram_tensor("idx_t", [n_chunks, 128, IDXW], i16, kind="ExternalInput")
    out_d = nc.dram_tensor("out", [R_PER_CORE, R_HIDDEN], f32, kind="ExternalOutput")
    tabH = nc.dram_tensor("tabH", [n_pad, ROW], bf16)
    tabT = nc.dram_tensor("tabT", [n_pad, ROW], bf16)

    p1_chunk = P1_TILES * 128
    n1_chunks = (n_pad + p1_chunk - 1) // p1_chunk

    with tile.TileContext(nc) as tc:
        with tc.tile_pool(name="const", bufs=1) as cp, \
             tc.tile_pool(name="psum_u", bufs=1, space="PSUM") as up:
            ident = cp.tile([128, 128], f32)
            make_identity(nc, ident[:])
            one11 = cp.tile([1, 1], f32)
            nc.vector.memset(one11[:], 1.0)

            U1T = up.tile([128, 128], f32)
            U2T = up.tile([128, 128], f32)
            z1T = up.tile([1, 128], f32)
            z2T = up.tile([1, 128], f32)

            # ---------------- Phase 1: node tables ----------------
            # valid (real-node) rows are [1, N_NODES+1)
            with tc.tile_pool(name="p1w", bufs=1) as p1w, \
                 tc.tile_pool(name="p1", bufs=3) as p1, \
                 tc.tile_pool(name="p1ps", bufs=4, space="PSUM") as p1ps:
                w0 = p1w.tile([128, 264], bf16)
                nc.sync.dma_start(out=w0[:], in_=wcat0_d[:])
                w1 = p1w.tile([128, 264], bf16)
                nc.sync.dma_start(out=w1[:], in_=wcat1_d[:])

                for ch in range(n1_chunks):
                    n0 = ch * p1_chunk
                    nn = min(p1_chunk, n_pad - n0)
                    nt = nn // 128
                    xk0 = p1.tile([128, p1_chunk], bf16, tag="xk0")
                    nc.sync.dma_start(out=xk0[:, :nn], in_=x_eT_d[0:128, n0:n0 + nn])
                    xk1 = p1.tile([128, p1_chunk], bf16, tag="xk1")
                    nc.sync.dma_start(out=xk1[:, :nn], in_=x_eT_d[128:256, n0:n0 + nn])
                    stage = p1.tile([128, P1_TILES, 2 * ROW], bf16, tag="stage")
                    for t in range(nt):
                        pp = p1ps.tile([128, 264], f32)
                        nc.tensor.matmul(
                            out=pp[:],
                            lhsT=xk0[:, t * 128:(t + 1) * 128],
                            rhs=w0[:], start=True, stop=False)
                        nc.tensor.matmul(
                            out=pp[:],
                            lhsT=xk1[:, t * 128:(t + 1) * 128],
                            rhs=w1[:], start=False, stop=True)
                        # H half: feats+ones-slot+s_a+s_b -> cols 0:131
                        nc.vector.tensor_copy(out=stage[:, t, 0:131],
                                              in_=pp[:, 0:131])
                        # T half -> cols ROW:ROW+131
                        nc.vector.tensor_copy(out=stage[:, t, ROW:ROW + 131],
                                              in_=pp[:, 132:263])
                        # ones column, only on real-node rows [1, N_NODES+1)
                        row0 = n0 + t * 128
                        p_hi = min(128, max(0, N_NODES + 1 - row0))
                        p_lo = max(0, 1 - row0)
                        for base in (C_ONE, ROW + C_ONE):
                            if p_hi > 0:
                                nc.vector.memset(
                                    stage[0:p_hi, t, base:base + 1], 1.0)
                            if p_lo > 0:
                                nc.vector.memset(
                                    stage[0:p_lo, t, base:base + 1], 0.0)
                    nc.gpsimd.memset(stage[:, :nt, 131:ROW], 0.0)
                    nc.gpsimd.memset(stage[:, :nt, ROW + 131:2 * ROW], 0.0)
                    nc.sync.dma_start(
                        out=tabH[n0:n0 + nn, :].rearrange("(t p) f -> p t f", p=128),
                        in_=stage[:, :nt, 0:ROW])
                    nc.sync.dma_start(
                        out=tabT[n0:n0 + nn, :].rearrange("(t p) f -> p t f", p=128),
                        in_=stage[:, :nt, ROW:2 * ROW])

            # ---------------- Phase 2: edge aggregation ----------------
            with tc.tile_pool(name="p2i", bufs=3) as p2i, \
                 tc.tile_pool(name="p2r", bufs=3) as p2r, \
                 tc.tile_pool(name="p2s", bufs=3) as p2s:
                for ch in range(n_chunks):
                    ih = p2i.tile([128, IDXW], i16, tag="ih")
                    nc.sync.dma_start(out=ih[:], in_=idxh_d[ch])
                    it = p2i.tile([128, IDXW], i16, tag="it")
                    nc.sync.dma_start(out=it[:], in_=idxt_d[ch])
                    rh = p2r.tile([128, T_C, ROW], bf16, tag="rh")
                    rt = p2r.tile([128, T_C, ROW], bf16, tag="rt")
                    # dma_gather is capped at 1024 indices/call (Q7 scratch
                    # + descriptor-ring limits) -> GSUB sub-calls per chunk
                    for s in range(GSUB):
                        t0 = s * (T_C // GSUB)
                        t1 = (s + 1) * (T_C // GSUB)
                        w0_ = s * (IDXW // GSUB)
                        w1_ = (s + 1) * (IDXW // GSUB)
                        nc.gpsimd.dma_gather(
                            out_ap=rh[:, t0:t1, :],
                            in_ap=tabH[int(chunk_hlo[ch]):, :],
                            idxs_ap=ih[:, w0_:w1_], num_idxs=GN,
                            num_idxs_reg=GN, elem_size=ROW)
                        nc.gpsimd.dma_gather(
                            out_ap=rt[:, t0:t1, :],
                            in_ap=tabT[int(chunk_tlo[ch]):, :],
                            idxs_ap=it[:, w0_:w1_], num_idxs=GN,
                            num_idxs_reg=GN, elem_size=ROW)

                    e1 = p2s.tile([128, T_C], f32, tag="e1")
                    nc.vector.tensor_tensor(
                        out=e1[:], in0=rh[:, :, C_SA], in1=rt[:, :, C_SA],
                        op=ALU.add)
                    nc.vector.scalar_tensor_tensor(
                        out=e1[:], in0=e1[:], scalar=0.01, in1=e1[:],
                        op0=ALU.mult, op1=ALU.max)
                    e2 = p2s.tile([128, T_C], f32, tag="e2")
                    nc.vector.tensor_tensor(
                        out=e2[:], in0=rh[:, :, C_SB], in1=rt[:, :, C_SB],
                        op=ALU.add)
                    nc.vector.scalar_tensor_tensor(
                        out=e2[:], in0=e2[:], scalar=0.01, in1=e2[:],
                        op0=ALU.mult, op1=ALU.max)
                    ex1 = p2s.tile([128, T_C], bf16, tag="ex1")
                    nc.scalar.activation(out=ex1[:], in_=e1[:], func=ACTF.Exp)
                    ex2 = p2s.tile([128, T_C], bf16, tag="ex2")
                    nc.scalar.activation(out=ex2[:], in_=e2[:], func=ACTF.Exp)

                    for j in range(T_C):
                        g = ch * T_C + j
                        sl = int(slot_of_tile[g])
                        st = bool(start_flag[g])
                        sp = bool(stop_flag[g])
                        nc.tensor.matmul(
                            out=U1T[:, sl:sl + 1], lhsT=rh[:, j, 0:128],
                            rhs=ex1[:, j:j + 1], start=st, stop=sp,
                            skip_group_check=True)
                        nc.tensor.matmul(
                            out=z1T[:, sl:sl + 1], lhsT=rh[:, j, C_ONE:C_ONE + 1],
                            rhs=ex1[:, j:j + 1], start=st, stop=sp,
                            skip_group_check=True)
                        nc.tensor.matmul(
                            out=U2T[:, sl:sl + 1], lhsT=rt[:, j, 0:128],
                            rhs=ex2[:, j:j + 1], start=st, stop=sp,
                            skip_group_check=True)
                        nc.tensor.matmul(
                            out=z2T[:, sl:sl + 1], lhsT=rt[:, j, C_ONE:C_ONE + 1],
                            rhs=ex2[:, j:j + 1], start=st, stop=sp,
                            skip_group_check=True)

            # ---------------- Finalize ----------------
            with tc.tile_pool(name="fin", bufs=2) as fp, \
                 tc.tile_pool(name="finps", bufs=2, space="PSUM") as fps:
                o_parts = []
                for UT, zT in ((U1T, z1T), (U2T, z2T)):
                    zrow = fp.tile([1, R_PER_CORE], f32, tag=None)
                    nc.vector.tensor_copy(out=zrow[:], in_=zT[0:1, 0:R_PER_CORE])
                    zcol_ps = fps.tile([R_PER_CORE, 1], f32, tag=None)
                    nc.tensor.matmul(out=zcol_ps[:], lhsT=zrow[:], rhs=one11[:],
                                     start=True, stop=True)
                    Usb = fp.tile([128, R_PER_CORE], f32, tag=None)
                    nc.vector.tensor_copy(out=Usb[:], in_=UT[:, 0:R_PER_CORE])
                    Tps = fps.tile([R_PER_CORE, 128], f32, tag=None)
                    nc.tensor.matmul(out=Tps[:], lhsT=Usb[:], rhs=ident[:],
                                     start=True, stop=True)
                    zc = fp.tile([R_PER_CORE, 1], f32, tag=None)
                    nc.vector.tensor_scalar(
                        out=zc[:], in0=zcol_ps[:], scalar1=1e-16, scalar2=None,
                        op0=ALU.add)
                    rr = fp.tile([R_PER_CORE, 1], f32, tag=None)
                    nc.vector.reciprocal(out=rr[:], in_=zc[:])
                    o = fp.tile([R_PER_CORE, R_HIDDEN], f32, tag=None)
                    nc.vector.tensor_scalar(
                        out=o[:], in0=Tps[:], scalar1=rr[:], scalar2=None,
                        op0=ALU.mult)
                    o_parts.append(o)
                nc.vector.tensor_tensor(out=o_parts[0][:], in0=o_parts[0][:],
                                        in1=o_parts[1][:], op=ALU.add)
                nc.sync.dma_start(out=out_d[:], in_=o_parts[0][:])

    nc.compile()
    return nc


def _wrap16(flat):
    """[4096] int16 gather-index list -> [128, 256] SBUF wrapped layout
    (16-partition wrap, replicated 8x across partition groups)."""
    w = flat.reshape(-1, 16).T  # [16, n/16]
    return np.tile(w, (8, 1))


def _prepare(x_e, edge_index, rel, W_h, W_t, a_h1, a_h2, a_t1, a_t2):
    """Host-side sharding/scheduling. Returns (in_maps, meta)."""
    import ml_dtypes
    bf = ml_dtypes.bfloat16

    h_idx = np.asarray(edge_index[0], dtype=np.int64)
    t_idx = np.asarray(edge_index[1], dtype=np.int64)
    rel = np.asarray(rel, dtype=np.int64)
    x_e = np.ascontiguousarray(np.asarray(x_e, dtype=np.float32))

    # table rows: 0 = zero row, 1..N = nodes, N+1.. = zero rows
    n_pad = ((N_NODES + 2 + 127) // 128) * 128
    zrow_hi = N_NODES + 1              # zero row reachable from the high half
    assert zrow_hi >= HALF and n_pad - HALF <= 32768 and HALF <= 32768

    x_eT = np.zeros((E_HIDDEN, n_pad), dtype=bf)
    x_eT[:, 1:N_NODES + 1] = x_e.T.astype(bf)

    # Augmented weights: [W | 0 (ones slot) | W@a1 | W@a2 | 0], H and T stacked.
    def aug(W, a1, a2):
        w = np.zeros((E_HIDDEN, 132), dtype=np.float32)
        w[:, :R_HIDDEN] = W
        w[:, C_SA] = W @ a1
        w[:, C_SB] = W @ a2
        return w

    w_h = aug(np.asarray(W_h, np.float32), np.asarray(a_h1, np.float32),
              np.asarray(a_t1, np.float32))
    w_t = aug(np.asarray(W_t, np.float32), np.asarray(a_h2, np.float32),
              np.asarray(a_t2, np.float32))
    wcat = np.concatenate([w_h, w_t], axis=1).astype(bf)  # [256, 264]
    wcat0 = np.ascontiguousarray(wcat[0:128])
    wcat1 = np.ascontiguousarray(wcat[128:256])

    # Edge partition by relation.
    counts = np.bincount(rel, minlength=R_RELS).astype(np.int64)
    order = np.argsort(rel, kind="stable")
    starts = np.zeros(R_RELS + 1, dtype=np.int64)
    starts[1:] = np.cumsum(counts)

    rel_by_slot = np.zeros((N_CORES, R_PER_CORE), dtype=np.int64)
    for c in range(N_CORES):
        rels = np.arange(R_PER_CORE * c, R_PER_CORE * (c + 1))
        rel_by_slot[c] = rels[np.argsort(-counts[rels], kind="stable")]

    # 4-way class split per (core, slot): cls = 2*(row_h>=HALF) + (row_t>=HALF)
    row_h = h_idx + 1
    row_t = t_idx + 1
    cls_of_edge = (row_h >= HALF).astype(np.int64) * 2 + (row_t >= HALF)

    cell_eids = {}                     # (core, cls, slot) -> edge ids
    cell_tiles = np.zeros((N_CORES, 4, R_PER_CORE), dtype=np.int64)
    for c in range(N_CORES):
        for j in range(R_PER_CORE):
            r = rel_by_slot[c, j]
            eids = order[starts[r]:starts[r + 1]]
            ecls = cls_of_edge[eids]
            for cls in range(4):
                sel = eids[ecls == cls]
                cell_eids[(c, cls, j)] = sel
                cell_tiles[c, cls, j] = (len(sel) + 127) // 128

    T_cell = cell_tiles.max(axis=0)    # [4, R_PER_CORE] shared schedule

    # Tile order: class-major blocks, each padded to a T_C multiple.
    tiles = []                         # (cls, slot or JUNK)
    for cls in range(4):
        for j in range(R_PER_CORE):
            tiles += [(cls, j)] * int(T_cell[cls, j])
        while len(tiles) % T_C:
            tiles.append((cls, JUNK_SLOT))
    n_tiles_pad = len(tiles)
    n_chunks = n_tiles_pad // T_C

    # PSUM start=True zeroes the whole 2KB bank (zero region), so the whole
    # phase-2 stream is ONE accumulation group per bank: per-byte pending-zero
    # makes each column overwrite-on-first-write, accumulate after.
    slot_of_tile = np.array([j for _, j in tiles], dtype=np.int64)
    start_flag = np.zeros(n_tiles_pad, dtype=bool)
    stop_flag = np.zeros(n_tiles_pad, dtype=bool)
    start_flag[0] = True
    stop_flag[n_tiles_pad - 1] = True

    chunk_cls = np.array([tiles[ch * T_C][0] for ch in range(n_chunks)],
                         dtype=np.int64)
    for ch in range(n_chunks):
        assert all(t[0] == chunk_cls[ch] for t in tiles[ch * T_C:(ch + 1) * T_C])
    chunk_hlo = np.where(chunk_cls >= 2, HALF, 0)
    chunk_tlo = np.where(chunk_cls % 2 == 1, HALF, 0)

    # Per-core int16 gather-index arrays, 16-wrapped per chunk.
    IDXW = (T_C * 128) // 16
    idx_h_maps, idx_t_maps = [], []
    for c in range(N_CORES):
        flat_h = np.zeros(n_tiles_pad * 128, dtype=np.int64)
        flat_t = np.zeros(n_tiles_pad * 128, dtype=np.int64)
        # defaults: zero row of the chunk's class half
        for g, (cls, j) in enumerate(tiles):
            flat_h[g * 128:(g + 1) * 128] = zrow_hi - HALF if cls >= 2 else 0
            flat_t[g * 128:(g + 1) * 128] = zrow_hi - HALF if cls % 2 else 0
        # fill real edges
        fill_pos = {}
        for g, (cls, j) in enumerate(tiles):
            if j == JUNK_SLOT:
                continue
            k = fill_pos.get((cls, j), 0)
            eids = cell_eids[(c, cls, j)][k * 128:(k + 1) * 128]
            fill_pos[(cls, j)] = k + 1
            if len(eids):
                hbase = HALF if cls >= 2 else 0
                tbase = HALF if cls % 2 else 0
                flat_h[g * 128:g * 128 + len(eids)] = row_h[eids] - hbase
                flat_t[g * 128:g * 128 + len(eids)] = row_t[eids] - tbase
        assert flat_h.max() < 32768 and flat_h.min() >= 0
        assert flat_t.max() < 32768 and flat_t.min() >= 0
        GN = min(1024, T_C * 128)
        gsub = (T_C * 128) // GN
        gw = GN // 16
        ih = np.empty((n_chunks, 128, IDXW), dtype=np.int16)
        itt = np.empty((n_chunks, 128, IDXW), dtype=np.int16)
        for ch in range(n_chunks):
            for s in range(gsub):
                o = ch * T_C * 128 + s * GN
                ih[ch, :, s * gw:(s + 1) * gw] = _wrap16(
                    flat_h[o:o + GN].astype(np.int16))
                itt[ch, :, s * gw:(s + 1) * gw] = _wrap16(
                    flat_t[o:o + GN].astype(np.int16))
        idx_h_maps.append(ih)
        idx_t_maps.append(itt)

    in_maps = []
    for c in range(N_CORES):
        in_maps.append({
            "x_eT": x_eT,
            "wcat0": wcat0,
            "wcat1": wcat1,
            "idx_h": idx_h_maps[c],
            "idx_t": idx_t_maps[c],
        })

    meta = {
        "n_pad": n_pad,
        "n_chunks": n_chunks,
        "chunk_hlo": chunk_hlo,
        "chunk_tlo": chunk_tlo,
        "slot_of_tile": slot_of_tile,
        "start_flag": start_flag,
        "stop_flag": stop_flag,
        "rel_by_slot": rel_by_slot,
    }
    return in_maps, meta


def kernel(x_e, edge_index, rel, W_h, W_t, a_h1, a_h2, a_t1, a_t2):
    from concourse.bass_utils import run_bass_kernel_spmd

    in_maps, meta = _prepare(x_e, edge_index, rel, W_h, W_t,
                             a_h1, a_h2, a_t1, a_t2)

    key = (meta["n_pad"], meta["n_chunks"],
           meta["chunk_hlo"].tobytes(), meta["chunk_tlo"].tobytes(),
           meta["slot_of_tile"].tobytes(),
           meta["start_flag"].tobytes(), meta["stop_flag"].tobytes())
    nc = _CACHE.get(key)
    if nc is None:
        nc = _build_program(meta["n_pad"], meta["n_chunks"],
                            meta["chunk_hlo"], meta["chunk_tlo"],
                            meta["slot_of_tile"], meta["start_flag"],
                            meta["stop_flag"])
        _CACHE.clear()
        _CACHE[key] = nc

    trace = os.environ.get("GAT_TRACE", "0") == "1"
    try:
        res = run_bass_kernel_spmd(nc, in_maps, list(range(N_CORES)), trace=trace)
    except Exception:
        if not trace:
            raise
        # profiling hook unavailable in this environment — run untraced
        res = run_bass_kernel_spmd(nc, in_maps, list(range(N_CORES)), trace=False)

    LAST_RUN_INFO.clear()
    LAST_RUN_INFO["exec_time_ns"] = res.exec_time_ns
    LAST_RUN_INFO["profile_json"] = res.profile_json

    out_full = np.zeros((R_RELS, R_HIDDEN), dtype=np.float32)
    rel_by_slot = meta["rel_by_slot"]
    for c in range(N_CORES):
        out_full[rel_by_slot[c]] = res.results[c]["out"]
    return out_full
